# revision 9
# baseline (speedup 1.0000x reference)
"""GQA attention kernel for Trainium2, 8-core SPMD.

Sharding: tensor-parallel=4 over kv-head pairs x data-parallel=2 over batch.
Each core: one batch, 8 q-heads, 2 kv-heads, full 2048-token sequence.
Host pre-transposes activations to [hidden, seq] so every matmul is native:
  - Q/K projections produce [d, s] (rope applied in-place via a PE
    half-swap permutation matmul + DVE combine with sign-folded sin table)
  - scores^T [k, q] = K_tile^T @ Q  (softmax reduction over partitions via
    ones-matmul on PE; exp on ACT directly out of PSUM with fused 1/sqrt(d)
    scale; no max-subtraction needed since |score| <~ 10)
  - attn^T [d, q] = V_tile^T @ exp  accumulated over k-chunks in PSUM
  - O partial = attn^T stacked as [f, q] feeding row-sharded Wo
Host sums the 4 TP partials per batch.
All matmuls bf16 inputs / fp32 PSUM accumulation.
"""
import numpy as np
import ml_dtypes

import concourse.bacc as bacc
import concourse.bass as bass
import concourse.tile as tile
from concourse import mybir
from concourse.bass_utils import run_bass_kernel_spmd

BF = mybir.dt.bfloat16
F32 = mybir.dt.float32
BF_NP = np.dtype(ml_dtypes.bfloat16)

# full-problem constants
B, S, HIDDEN = 2, 2048, 4096
NUM_HEADS, NUM_KV_HEADS, HEAD_DIM = 32, 8, 128
GROUPS = NUM_HEADS // NUM_KV_HEADS
ROPE_THETA = 10000.0
TP = 4  # shards over kv-head pairs

FULL_CFG = dict(S=2048, HID=4096, NQ=8, NKV=2, SB=512, QC=512)


def build_nc(cfg):
    S_, HID, NQ, NKV, SB, QC = (cfg[k] for k in ("S", "HID", "NQ", "NKV", "SB", "QC"))
    D = 128
    HC = HID // 128          # hidden chunks (contraction tiles)
    NB = S_ // SB            # phase-1 token blocks
    NQC = S_ // QC           # attention q chunks
    KT = S_ // 128           # k-token tiles
    DV = NKV * 128           # local v width
    NO = HID // 512          # O-proj output chunks
    scale = 1.0 / np.sqrt(128.0)

    nc = bacc.Bacc("TRN2", target_bir_lowering=False, debug=False)
    xt = nc.dram_tensor("xt", (HID, S_), BF, kind="ExternalInput").ap()
    wq = nc.dram_tensor("wq", (HC, NQ, 128, 128), BF, kind="ExternalInput").ap()
    wk = nc.dram_tensor("wk", (HC, NKV, 128, 128), BF, kind="ExternalInput").ap()
    wv = nc.dram_tensor("wv", (HC, 128, DV), BF, kind="ExternalInput").ap()
    wo = nc.dram_tensor("wo", (NQ, NO, 128, 512), BF, kind="ExternalInput").ap()
    cosd = nc.dram_tensor("cos", (128, S_), BF, kind="ExternalInput").ap()
    sind = nc.dram_tensor("sin", (128, S_), BF, kind="ExternalInput").ap()
    rmatd = nc.dram_tensor("rmat", (128, 128), BF, kind="ExternalInput").ap()
    o = nc.dram_tensor("o", (S_, HID), F32, kind="ExternalOutput").ap()

    with tile.TileContext(nc) as tc:
        with tc.tile_pool(name="cons", bufs=1) as cons, \
             tc.tile_pool(name="big", bufs=1) as big:
            cos_sb = cons.tile([128, S_], BF, name="cos_sb")
            sin_sb = cons.tile([128, S_], BF, name="sin_sb")
            r_sb = cons.tile([128, 128], BF, name="r_sb")
            ones_sb = cons.tile([128, 1], BF, name="ones_sb")
            nc.sync.dma_start(out=cos_sb, in_=cosd)
            nc.sync.dma_start(out=sin_sb, in_=sind)
            nc.sync.dma_start(out=r_sb, in_=rmatd)
            nc.vector.memset(ones_sb, 1.0)

            q_sb = big.tile([128, NQ, S_], BF, name="q_sb")
            k_sb = big.tile([128, NKV, S_], BF, name="k_sb")
            v_sb = big.tile([128, KT, DV], BF, name="v_sb")
            wv_sb = big.tile([128, HC, DV], BF, name="wv_sb")
            nc.sync.dma_start(out=wv_sb, in_=wv.rearrange("c p v -> p c v"))

            xt_r = xt.rearrange("(c p) s -> p c s", p=128)

            # ---------------- phase 1: projections + rope ----------------
            with tc.tile_pool(name="xp", bufs=2) as xp, \
                 tc.tile_pool(name="wp", bufs=8) as wp, \
                 tc.tile_pool(name="rt", bufs=4) as rt, \
                 tc.tile_pool(name="pp", bufs=2, space="PSUM") as pp, \
                 tc.tile_pool(name="rp", bufs=2, space="PSUM") as rp:
                for sb_i in range(NB):
                    ssl = slice(sb_i * SB, (sb_i + 1) * SB)
                    xt_t = xp.tile([128, HC, SB], BF, name="xt_t")
                    nc.sync.dma_start(out=xt_t, in_=xt_r[:, :, ssl])

                    # Q then K projections, each with rope
                    for which, nheads, wten, dst in (
                        ("q", NQ, wq, q_sb), ("k", NKV, wk, k_sb)):
                        for h in range(nheads):
                            ps = pp.tile([128, SB], F32, name="ps_proj")
                            for c in range(HC):
                                wt = wp.tile([128, 128], BF, name="w_t")
                                nc.sync.dma_start(out=wt, in_=wten[c, h])
                                nc.tensor.matmul(ps, wt, xt_t[:, c, :],
                                                 start=(c == 0), stop=(c == HC - 1))
                            # rope: out = ps*cos + (R@ps)*sin_signed
                            qbf = rt.tile([128, SB], BF, name="rope_bf")
                            nc.scalar.activation(out=qbf, in_=ps,
                                                 func=mybir.ActivationFunctionType.Copy)
                            rot = rp.tile([128, SB], F32, name="rot_ps")
                            nc.tensor.matmul(rot, r_sb, qbf, start=True, stop=True)
                            t1 = rt.tile([128, SB], F32, name="rope_t1")
                            t2 = rt.tile([128, SB], F32, name="rope_t2")
                            nc.vector.tensor_mul(t1, ps, cos_sb[:, ssl])
                            nc.vector.tensor_mul(t2, rot, sin_sb[:, ssl])
                            nc.vector.tensor_add(dst[:, h, ssl], t1, t2)

                    # V projection (natural [tok, d] layout)
                    for tt in range(SB // 128):
                        ps = pp.tile([128, DV], F32, name="ps_v")
                        for c in range(HC):
                            nc.tensor.matmul(ps, xt_t[:, c, tt * 128:(tt + 1) * 128],
                                             wv_sb[:, c, :],
                                             start=(c == 0), stop=(c == HC - 1))
                        nc.scalar.activation(out=v_sb[:, sb_i * (SB // 128) + tt, :],
                                             in_=ps,
                                             func=mybir.ActivationFunctionType.Copy)

            # ------------- phase 2+3: attention + output projection -------------
            with tc.tile_pool(name="aq", bufs=2) as aq, \
                 tc.tile_pool(name="ep", bufs=3) as ep, \
                 tc.tile_pool(name="rb", bufs=2) as rb, \
                 tc.tile_pool(name="ob", bufs=3) as ob, \
                 tc.tile_pool(name="wob", bufs=16) as wob, \
                 tc.tile_pool(name="dsc", bufs=2, space="DRAM") as dsc, \
                 tc.tile_pool(name="sp", bufs=2, space="PSUM") as sp, \
                 tc.tile_pool(name="ap_", bufs=2, space="PSUM") as ap_, \
                 tc.tile_pool(name="dp", bufs=2, space="PSUM") as dp, \
                 tc.tile_pool(name="op", bufs=2, space="PSUM") as op:
                for qc in range(NQC):
                    qsl = slice(qc * QC, (qc + 1) * QC)
                    at_qc = aq.tile([128, NQ, QC], BF, name="at_qc")
                    for h in range(NQ):
                        kvh = h // (NQ // NKV)
                        attn_ps = ap_.tile([128, QC], F32, name="attn_ps")
                        den_ps = dp.tile([1, QC], F32, name="den_ps")
                        for kc in range(KT):
                            s_ps = sp.tile([128, QC], F32, name="s_ps")
                            nc.tensor.matmul(
                                s_ps, k_sb[:, kvh, kc * 128:(kc + 1) * 128],
                                q_sb[:, h, qsl], start=True, stop=True)
                            e_t = ep.tile([128, QC], BF, name="e_t")
                            nc.scalar.activation(
                                out=e_t, in_=s_ps,
                                func=mybir.ActivationFunctionType.Exp, scale=scale)
                            nc.tensor.matmul(
                                attn_ps, v_sb[:, kc, kvh * 128:(kvh + 1) * 128], e_t,
                                start=(kc == 0), stop=(kc == KT - 1),
                                skip_group_check=True)
                            nc.tensor.matmul(
                                den_ps, ones_sb, e_t,
                                start=(kc == 0), stop=(kc == KT - 1),
                                skip_group_check=True)
                        rec = rb.tile([1, QC], F32, name="rec")
                        nc.vector.reciprocal(out=rec, in_=den_ps)
                        rec_dram = dsc.tile([1, QC], F32, name="rec_dram")
                        nc.sync.dma_start(out=rec_dram, in_=rec)
                        rec_bc = rb.tile([128, QC], F32, name="rec_bc")
                        nc.sync.dma_start(
                            out=rec_bc,
                            in_=bass.AP(tensor=rec_dram.tensor, offset=rec_dram.offset,
                                        ap=[[0, 128]] + list(rec_dram.ap[1:])))
                        nc.vector.tensor_mul(at_qc[:, h, :], attn_ps, rec_bc)

                    # output projection for this q-chunk
                    for n in range(NO):
                        wo_ts = []
                        for c in range(NQ):
                            wot = wob.tile([128, 512], BF, name="wo_t")
                            nc.sync.dma_start(out=wot, in_=wo[c, n])
                            wo_ts.append(wot)
                        for tt in range(QC // 128):
                            tok0 = qc * QC + tt * 128
                            ps_o = op.tile([128, 512], F32, name="ps_o")
                            for c in range(NQ):
                                nc.tensor.matmul(
                                    ps_o, at_qc[:, c, tt * 128:(tt + 1) * 128],
                                    wo_ts[c],
                                    start=(c == 0), stop=(c == NQ - 1))
                            o_t = ob.tile([128, 512], F32, name="o_t")
                            nc.scalar.activation(out=o_t, in_=ps_o,
                                                 func=mybir.ActivationFunctionType.Copy)
                            nc.sync.dma_start(
                                out=o[tok0:tok0 + 128, n * 512:(n + 1) * 512],
                                in_=o_t)
                        del wo_ts
    nc.compile()
    return nc


def _rope_tables(position_ids_b, S_):
    """cos/sin tables in [d=128, s] layout, sin sign-folded for the half-swap."""
    pos = position_ids_b.astype(np.float32)
    inv_freq = (1.0 / (ROPE_THETA ** (np.arange(0, HEAD_DIM, 2, dtype=np.float32)
                                      / HEAD_DIM))).astype(np.float32)
    freqs = pos[:, None] * inv_freq[None, :]          # [s, 64]
    emb = np.concatenate([freqs, freqs], axis=1)      # [s, 128]
    cos = np.cos(emb).T.copy()                        # [128, s]
    sin = np.sin(emb).T.copy()
    sin[:64] *= -1.0                                  # sign-fold for swap rope
    return cos.astype(BF_NP), sin.astype(BF_NP)


def _prep_core_inputs(hidden_states, position_ids, Wq, Wk, Wv, Wo):
    rmat = np.zeros((128, 128), dtype=np.float32)
    for i in range(128):
        rmat[i, (i + 64) % 128] = 1.0
    rmat = rmat.astype(BF_NP)

    HC = HIDDEN // 128
    in_maps = []
    for t in range(TP):
        fq = slice(1024 * t, 1024 * (t + 1))
        fkv = slice(256 * t, 256 * (t + 1))
        wq_t = np.ascontiguousarray(
            Wq[:, fq].reshape(HC, 128, 8, 128).transpose(0, 2, 1, 3)).astype(BF_NP)
        wk_t = np.ascontiguousarray(
            Wk[:, fkv].reshape(HC, 128, 2, 128).transpose(0, 2, 1, 3)).astype(BF_NP)
        wv_t = np.ascontiguousarray(Wv[:, fkv].reshape(HC, 128, 256)).astype(BF_NP)
        wo_t = np.ascontiguousarray(
            Wo[fq, :].reshape(8, 128, 8, 512).transpose(0, 2, 1, 3)).astype(BF_NP)
        for b in range(B):
            xt = np.ascontiguousarray(hidden_states[b].T).astype(BF_NP)
            cos, sin = _rope_tables(position_ids[b], S)
            in_maps.append({"xt": xt, "wq": wq_t, "wk": wk_t, "wv": wv_t,
                            "wo": wo_t, "cos": cos, "sin": sin, "rmat": rmat})
    return in_maps


_NC_CACHE = {}


def kernel(hidden_states, position_ids, Wq, Wk, Wv, Wo):
    if "nc" not in _NC_CACHE:
        _NC_CACHE["nc"] = build_nc(FULL_CFG)
    nc = _NC_CACHE["nc"]
    in_maps = _prep_core_inputs(np.asarray(hidden_states), np.asarray(position_ids),
                                np.asarray(Wq), np.asarray(Wk),
                                np.asarray(Wv), np.asarray(Wo))
    res = run_bass_kernel_spmd(nc, in_maps, core_ids=list(range(8)))
    out = np.zeros((B, S, HIDDEN), dtype=np.float32)
    for t in range(TP):
        for b in range(B):
            out[b] += res.results[t * B + b]["o"]
    return out


# revision 13
# speedup vs baseline: 1.5293x; 1.5293x over previous
"""GQA attention kernel for Trainium2, 8-core SPMD.

Sharding: tensor-parallel=4 over kv-head pairs x data-parallel=2 over batch.
Each core: one batch, 8 q-heads, 2 kv-heads, full 2048-token sequence.
Host pre-transposes activations to [hidden, seq] so every matmul is native:
  - Q/K projections produce [d, s] (rope applied in-place via a PE
    half-swap permutation matmul + DVE combine with sign-folded sin table)
  - scores^T [k, q] = K_tile^T @ Q  (softmax reduction over partitions via
    ones-matmul on PE; exp on ACT directly out of PSUM with fused 1/sqrt(d)
    scale; no max-subtraction needed since |score| <~ 10)
  - attn^T [d, q] = V_tile^T @ exp  accumulated over k-chunks in PSUM
  - O partial = attn^T stacked as [f, q] feeding row-sharded Wo
Host sums the 4 TP partials per batch.
All matmuls bf16 inputs / fp32 PSUM accumulation.
"""
import numpy as np
import ml_dtypes

import concourse.bacc as bacc
import concourse.bass as bass
import concourse.tile as tile
from concourse import mybir
from concourse.bass_utils import run_bass_kernel_spmd

BF = mybir.dt.bfloat16
F32 = mybir.dt.float32
BF_NP = np.dtype(ml_dtypes.bfloat16)

# full-problem constants
B, S, HIDDEN = 2, 2048, 4096
NUM_HEADS, NUM_KV_HEADS, HEAD_DIM = 32, 8, 128
GROUPS = NUM_HEADS // NUM_KV_HEADS
ROPE_THETA = 10000.0
TP = 4  # shards over kv-head pairs

FULL_CFG = dict(S=2048, HID=4096, NQ=8, NKV=2, SB=512, QC=512)


def build_nc(cfg):
    S_, HID, NQ, NKV, SB, QC = (cfg[k] for k in ("S", "HID", "NQ", "NKV", "SB", "QC"))
    D = 128
    HC = HID // 128          # hidden chunks (contraction tiles)
    NB = S_ // SB            # phase-1 token blocks
    NQC = S_ // QC           # attention q chunks
    KT = S_ // 128           # k-token tiles
    DV = NKV * 128           # local v width
    NO = HID // 512          # O-proj output chunks
    scale = 1.0 / np.sqrt(128.0)

    nc = bacc.Bacc("TRN2", target_bir_lowering=False, debug=False)
    xt = nc.dram_tensor("xt", (HID, S_), BF, kind="ExternalInput").ap()
    wq = nc.dram_tensor("wq", (HC, NQ, 128, 128), BF, kind="ExternalInput").ap()
    wk = nc.dram_tensor("wk", (HC, NKV, 128, 128), BF, kind="ExternalInput").ap()
    wv = nc.dram_tensor("wv", (HC, 128, DV), BF, kind="ExternalInput").ap()
    wo = nc.dram_tensor("wo", (NQ, NO, 128, 512), BF, kind="ExternalInput").ap()
    cosd = nc.dram_tensor("cos", (128, S_), BF, kind="ExternalInput").ap()
    sind = nc.dram_tensor("sin", (128, S_), BF, kind="ExternalInput").ap()
    rmatd = nc.dram_tensor("rmat", (128, 128), BF, kind="ExternalInput").ap()
    o = nc.dram_tensor("o", (S_, HID), F32, kind="ExternalOutput").ap()

    with tile.TileContext(nc) as tc:
        with tc.tile_pool(name="cons", bufs=1) as cons, \
             tc.tile_pool(name="big", bufs=1) as big:
            cos_sb = cons.tile([128, S_], BF, name="cos_sb")
            sin_sb = cons.tile([128, S_], BF, name="sin_sb")
            r_sb = cons.tile([128, 128], BF, name="r_sb")
            ones_sb = cons.tile([128, 1], BF, name="ones_sb")
            nc.sync.dma_start(out=cos_sb, in_=cosd)
            nc.sync.dma_start(out=sin_sb, in_=sind)
            nc.sync.dma_start(out=r_sb, in_=rmatd)
            nc.vector.memset(ones_sb, 1.0)

            q_sb = big.tile([128, NQ, S_], BF, name="q_sb")
            k_sb = big.tile([128, NKV, S_], BF, name="k_sb")
            v_sb = big.tile([128, KT, DV], BF, name="v_sb")
            wv_sb = big.tile([128, HC, DV], BF, name="wv_sb")
            nc.sync.dma_start(out=wv_sb, in_=wv.rearrange("c p v -> p c v"))

            xt_r = xt.rearrange("(c p) s -> p c s", p=128)

            # ---------------- phase 1: projections + rope ----------------
            with tc.tile_pool(name="xp", bufs=2) as xp, \
                 tc.tile_pool(name="wp", bufs=3) as wp, \
                 tc.tile_pool(name="rt", bufs=4) as rt, \
                 tc.tile_pool(name="pp", bufs=2, space="PSUM") as pp, \
                 tc.tile_pool(name="rp", bufs=2, space="PSUM") as rp:
                for sb_i in range(NB):
                    ssl = slice(sb_i * SB, (sb_i + 1) * SB)
                    xt_t = xp.tile([128, HC, SB], BF, name="xt_t")
                    nc.sync.dma_start(out=xt_t, in_=xt_r[:, :, ssl])

                    # Q then K projections, each with rope
                    for which, nheads, wten, dst in (
                        ("q", NQ, wq, q_sb), ("k", NKV, wk, k_sb)):
                        for h in range(nheads):
                            ps = pp.tile([128, SB], F32, name="ps_proj")
                            wslab = wp.tile([128, HC, 128], BF, name="w_slab")
                            nc.sync.dma_start(
                                out=wslab,
                                in_=wten[:, h].rearrange("c p m -> p c m"))
                            for c in range(HC):
                                nc.tensor.matmul(ps, wslab[:, c, :], xt_t[:, c, :],
                                                 start=(c == 0), stop=(c == HC - 1))
                            # rope: out = ps*cos + (R@ps)*sin_signed
                            qbf = rt.tile([128, SB], BF, name="rope_bf")
                            nc.scalar.activation(out=qbf, in_=ps,
                                                 func=mybir.ActivationFunctionType.Copy)
                            rot = rp.tile([128, SB], F32, name="rot_ps")
                            nc.tensor.matmul(rot, r_sb, qbf, start=True, stop=True)
                            t1 = rt.tile([128, SB], F32, name="rope_t1")
                            t2 = rt.tile([128, SB], F32, name="rope_t2")
                            nc.vector.tensor_mul(t1, ps, cos_sb[:, ssl])
                            nc.vector.tensor_mul(t2, rot, sin_sb[:, ssl])
                            nc.vector.tensor_add(dst[:, h, ssl], t1, t2)

                    # V projection (natural [tok, d] layout)
                    for tt in range(SB // 128):
                        ps = pp.tile([128, DV], F32, name="ps_v")
                        for c in range(HC):
                            nc.tensor.matmul(ps, xt_t[:, c, tt * 128:(tt + 1) * 128],
                                             wv_sb[:, c, :],
                                             start=(c == 0), stop=(c == HC - 1))
                        nc.scalar.activation(out=v_sb[:, sb_i * (SB // 128) + tt, :],
                                             in_=ps,
                                             func=mybir.ActivationFunctionType.Copy)

            # ------------- phase 2+3: attention + output projection -------------
            with tc.tile_pool(name="aq", bufs=2) as aq, \
                 tc.tile_pool(name="ep", bufs=3) as ep, \
                 tc.tile_pool(name="rb", bufs=2) as rb, \
                 tc.tile_pool(name="ob", bufs=2) as ob, \
                 tc.tile_pool(name="wob", bufs=3) as wob, \
                 tc.tile_pool(name="dsc", bufs=2, space="DRAM") as dsc, \
                 tc.tile_pool(name="sp", bufs=2, space="PSUM") as sp, \
                 tc.tile_pool(name="ap_", bufs=2, space="PSUM") as ap_, \
                 tc.tile_pool(name="dp", bufs=2, space="PSUM") as dp, \
                 tc.tile_pool(name="op", bufs=2, space="PSUM") as op:
                for qc in range(NQC):
                    qsl = slice(qc * QC, (qc + 1) * QC)
                    at_qc = aq.tile([128, NQ, QC], BF, name="at_qc")
                    for h in range(NQ):
                        kvh = h // (NQ // NKV)
                        attn_ps = ap_.tile([128, QC], F32, name="attn_ps")
                        den_ps = dp.tile([1, QC], F32, name="den_ps")
                        for kc in range(KT):
                            s_ps = sp.tile([128, QC], F32, name="s_ps")
                            nc.tensor.matmul(
                                s_ps, k_sb[:, kvh, kc * 128:(kc + 1) * 128],
                                q_sb[:, h, qsl], start=True, stop=True)
                            e_t = ep.tile([128, QC], BF, name="e_t")
                            nc.scalar.activation(
                                out=e_t, in_=s_ps,
                                func=mybir.ActivationFunctionType.Exp, scale=scale)
                            nc.tensor.matmul(
                                attn_ps, v_sb[:, kc, kvh * 128:(kvh + 1) * 128], e_t,
                                start=(kc == 0), stop=(kc == KT - 1),
                                skip_group_check=True)
                            nc.tensor.matmul(
                                den_ps, ones_sb, e_t,
                                start=(kc == 0), stop=(kc == KT - 1),
                                skip_group_check=True)
                        rec = rb.tile([1, QC], F32, name="rec")
                        nc.vector.reciprocal(out=rec, in_=den_ps)
                        rec_dram = dsc.tile([1, QC], F32, name="rec_dram")
                        nc.sync.dma_start(out=rec_dram, in_=rec)
                        rec_bc = rb.tile([128, QC], F32, name="rec_bc")
                        nc.sync.dma_start(
                            out=rec_bc,
                            in_=bass.AP(tensor=rec_dram.tensor, offset=rec_dram.offset,
                                        ap=[[0, 128]] + list(rec_dram.ap[1:])))
                        nc.vector.tensor_mul(at_qc[:, h, :], attn_ps, rec_bc)

                    # output projection for this q-chunk
                    for n in range(NO):
                        wos = wob.tile([128, NQ, 512], BF, name="wo_slab")
                        nc.sync.dma_start(
                            out=wos, in_=wo[:, n].rearrange("c p m -> p c m"))
                        o_t = ob.tile([128, QC // 128, 512], F32, name="o_t")
                        for tt in range(QC // 128):
                            ps_o = op.tile([128, 512], F32, name="ps_o")
                            for c in range(NQ):
                                nc.tensor.matmul(
                                    ps_o, at_qc[:, c, tt * 128:(tt + 1) * 128],
                                    wos[:, c, :],
                                    start=(c == 0), stop=(c == NQ - 1))
                            nc.scalar.activation(out=o_t[:, tt, :], in_=ps_o,
                                                 func=mybir.ActivationFunctionType.Copy)
                        nc.sync.dma_start(
                            out=o[qsl, n * 512:(n + 1) * 512].rearrange(
                                "(t p) m -> p t m", p=128),
                            in_=o_t)
    nc.compile()
    return nc


def _rope_tables(position_ids_b, S_):
    """cos/sin tables in [d=128, s] layout, sin sign-folded for the half-swap."""
    pos = position_ids_b.astype(np.float32)
    inv_freq = (1.0 / (ROPE_THETA ** (np.arange(0, HEAD_DIM, 2, dtype=np.float32)
                                      / HEAD_DIM))).astype(np.float32)
    freqs = pos[:, None] * inv_freq[None, :]          # [s, 64]
    emb = np.concatenate([freqs, freqs], axis=1)      # [s, 128]
    cos = np.cos(emb).T.copy()                        # [128, s]
    sin = np.sin(emb).T.copy()
    sin[:64] *= -1.0                                  # sign-fold for swap rope
    return cos.astype(BF_NP), sin.astype(BF_NP)


def _prep_core_inputs(hidden_states, position_ids, Wq, Wk, Wv, Wo):
    rmat = np.zeros((128, 128), dtype=np.float32)
    for i in range(128):
        rmat[i, (i + 64) % 128] = 1.0
    rmat = rmat.astype(BF_NP)

    HC = HIDDEN // 128
    in_maps = []
    for t in range(TP):
        fq = slice(1024 * t, 1024 * (t + 1))
        fkv = slice(256 * t, 256 * (t + 1))
        wq_t = np.ascontiguousarray(
            Wq[:, fq].reshape(HC, 128, 8, 128).transpose(0, 2, 1, 3)).astype(BF_NP)
        wk_t = np.ascontiguousarray(
            Wk[:, fkv].reshape(HC, 128, 2, 128).transpose(0, 2, 1, 3)).astype(BF_NP)
        wv_t = np.ascontiguousarray(Wv[:, fkv].reshape(HC, 128, 256)).astype(BF_NP)
        wo_t = np.ascontiguousarray(
            Wo[fq, :].reshape(8, 128, 8, 512).transpose(0, 2, 1, 3)).astype(BF_NP)
        for b in range(B):
            xt = np.ascontiguousarray(hidden_states[b].T).astype(BF_NP)
            cos, sin = _rope_tables(position_ids[b], S)
            in_maps.append({"xt": xt, "wq": wq_t, "wk": wk_t, "wv": wv_t,
                            "wo": wo_t, "cos": cos, "sin": sin, "rmat": rmat})
    return in_maps


_NC_CACHE = {}


def kernel(hidden_states, position_ids, Wq, Wk, Wv, Wo):
    if "nc" not in _NC_CACHE:
        _NC_CACHE["nc"] = build_nc(FULL_CFG)
    nc = _NC_CACHE["nc"]
    in_maps = _prep_core_inputs(np.asarray(hidden_states), np.asarray(position_ids),
                                np.asarray(Wq), np.asarray(Wk),
                                np.asarray(Wv), np.asarray(Wo))
    res = run_bass_kernel_spmd(nc, in_maps, core_ids=list(range(8)))
    out = np.zeros((B, S, HIDDEN), dtype=np.float32)
    for t in range(TP):
        for b in range(B):
            out[b] += res.results[t * B + b]["o"]
    return out


# revision 15
# speedup vs baseline: 1.5964x; 1.0439x over previous
"""GQA attention kernel for Trainium2, 8-core SPMD.

Sharding: tensor-parallel=4 over kv-head pairs x data-parallel=2 over batch.
Each core: one batch, 8 q-heads, 2 kv-heads, full 2048-token sequence.
Host pre-transposes activations to [hidden, seq] so every matmul is native:
  - Q/K projections produce [d, s] (rope applied in-place via a PE
    half-swap permutation matmul + DVE combine with sign-folded sin table)
  - scores^T [k, q] = K_tile^T @ Q  (softmax reduction over partitions via
    ones-matmul on PE; exp on ACT directly out of PSUM with fused 1/sqrt(d)
    scale; no max-subtraction needed since |score| <~ 10)
  - attn^T [d, q] = V_tile^T @ exp  accumulated over k-chunks in PSUM
  - O partial = attn^T stacked as [f, q] feeding row-sharded Wo
Host sums the 4 TP partials per batch.
All matmuls bf16 inputs / fp32 PSUM accumulation.
"""
import numpy as np
import ml_dtypes

import concourse.bacc as bacc
import concourse.bass as bass
import concourse.tile as tile
from concourse import mybir
from concourse.bass_utils import run_bass_kernel_spmd

BF = mybir.dt.bfloat16
F32 = mybir.dt.float32
BF_NP = np.dtype(ml_dtypes.bfloat16)

# full-problem constants
B, S, HIDDEN = 2, 2048, 4096
NUM_HEADS, NUM_KV_HEADS, HEAD_DIM = 32, 8, 128
GROUPS = NUM_HEADS // NUM_KV_HEADS
ROPE_THETA = 10000.0
TP = 4  # shards over kv-head pairs

FULL_CFG = dict(S=2048, HID=4096, NQ=8, NKV=2, SB=512, QC=512)


def build_nc(cfg):
    S_, HID, NQ, NKV, SB, QC = (cfg[k] for k in ("S", "HID", "NQ", "NKV", "SB", "QC"))
    D = 128
    HC = HID // 128          # hidden chunks (contraction tiles)
    NB = S_ // SB            # phase-1 token blocks
    NQC = S_ // QC           # attention q chunks
    KT = S_ // 128           # k-token tiles
    DV = NKV * 128           # local v width
    NO = HID // 512          # O-proj output chunks
    scale = 1.0 / np.sqrt(128.0)

    nc = bacc.Bacc("TRN2", target_bir_lowering=False, debug=False)
    xt = nc.dram_tensor("xt", (HID, S_), BF, kind="ExternalInput").ap()
    wq = nc.dram_tensor("wq", (HC, NQ, 128, 128), BF, kind="ExternalInput").ap()
    wk = nc.dram_tensor("wk", (HC, NKV, 128, 128), BF, kind="ExternalInput").ap()
    wv = nc.dram_tensor("wv", (HC, 128, DV), BF, kind="ExternalInput").ap()
    wo = nc.dram_tensor("wo", (NQ, NO, 128, 512), BF, kind="ExternalInput").ap()
    cosd = nc.dram_tensor("cos", (128, S_), BF, kind="ExternalInput").ap()
    sind = nc.dram_tensor("sin", (128, S_), BF, kind="ExternalInput").ap()
    rmatd = nc.dram_tensor("rmat", (128, 128), BF, kind="ExternalInput").ap()
    o = nc.dram_tensor("o", (S_, HID), F32, kind="ExternalOutput").ap()

    with tile.TileContext(nc) as tc:
        with tc.tile_pool(name="cons", bufs=1) as cons, \
             tc.tile_pool(name="big", bufs=1) as big:
            cos_sb = cons.tile([128, S_], BF, name="cos_sb")
            sin_sb = cons.tile([128, S_], BF, name="sin_sb")
            r_sb = cons.tile([128, 128], BF, name="r_sb")
            ones_sb = cons.tile([128, 1], BF, name="ones_sb")
            nc.sync.dma_start(out=cos_sb, in_=cosd)
            nc.sync.dma_start(out=sin_sb, in_=sind)
            nc.sync.dma_start(out=r_sb, in_=rmatd)
            nc.vector.memset(ones_sb, 1.0)

            q_sb = big.tile([128, NQ, S_], BF, name="q_sb")
            k_sb = big.tile([128, NKV, S_], BF, name="k_sb")
            v_sb = big.tile([128, KT, DV], BF, name="v_sb")
            wv_sb = big.tile([128, HC, DV], BF, name="wv_sb")
            nc.sync.dma_start(out=wv_sb, in_=wv.rearrange("c p v -> p c v"))

            xt_r = xt.rearrange("(c p) s -> p c s", p=128)

            # ---------------- phase 1: projections + rope ----------------
            with tc.tile_pool(name="xp", bufs=2) as xp, \
                 tc.tile_pool(name="wp", bufs=3) as wp, \
                 tc.tile_pool(name="rt", bufs=4) as rt, \
                 tc.tile_pool(name="pp", bufs=2, space="PSUM") as pp, \
                 tc.tile_pool(name="rp", bufs=2, space="PSUM") as rp:
                for sb_i in range(NB):
                    ssl = slice(sb_i * SB, (sb_i + 1) * SB)
                    xt_t = xp.tile([128, HC, SB], BF, name="xt_t")
                    nc.sync.dma_start(out=xt_t, in_=xt_r[:, :, ssl])

                    # Q then K projections, each with rope
                    for which, nheads, wten, dst in (
                        ("q", NQ, wq, q_sb), ("k", NKV, wk, k_sb)):
                        for h in range(nheads):
                            ps = pp.tile([128, SB], F32, name="ps_proj")
                            wslab = wp.tile([128, HC, 128], BF, name="w_slab")
                            nc.sync.dma_start(
                                out=wslab,
                                in_=wten[:, h].rearrange("c p m -> p c m"))
                            for c in range(HC):
                                nc.tensor.matmul(ps, wslab[:, c, :], xt_t[:, c, :],
                                                 start=(c == 0), stop=(c == HC - 1))
                            # rope: out = ps*cos + (R@ps)*sin_signed
                            qbf = rt.tile([128, SB], BF, name="rope_bf")
                            nc.scalar.activation(out=qbf, in_=ps,
                                                 func=mybir.ActivationFunctionType.Copy)
                            rot = rp.tile([128, SB], F32, name="rot_ps")
                            nc.tensor.matmul(rot, r_sb, qbf, start=True, stop=True)
                            t1 = rt.tile([128, SB], F32, name="rope_t1")
                            t2 = rt.tile([128, SB], F32, name="rope_t2")
                            nc.vector.tensor_mul(t1, ps, cos_sb[:, ssl])
                            nc.vector.tensor_mul(t2, rot, sin_sb[:, ssl])
                            nc.vector.tensor_add(dst[:, h, ssl], t1, t2)

                    # V projection (natural [tok, d] layout)
                    for tt in range(SB // 128):
                        ps = pp.tile([128, DV], F32, name="ps_v")
                        for c in range(HC):
                            nc.tensor.matmul(ps, xt_t[:, c, tt * 128:(tt + 1) * 128],
                                             wv_sb[:, c, :],
                                             start=(c == 0), stop=(c == HC - 1))
                        nc.scalar.activation(out=v_sb[:, sb_i * (SB // 128) + tt, :],
                                             in_=ps,
                                             func=mybir.ActivationFunctionType.Copy)

            # ------------- phase 2+3: attention + output projection -------------
            with tc.tile_pool(name="aq", bufs=2) as aq, \
                 tc.tile_pool(name="ep", bufs=3) as ep, \
                 tc.tile_pool(name="rb", bufs=2) as rb, \
                 tc.tile_pool(name="ob", bufs=2) as ob, \
                 tc.tile_pool(name="wob", bufs=3) as wob, \
                 tc.tile_pool(name="sp", bufs=3, space="PSUM") as sp, \
                 tc.tile_pool(name="ap_", bufs=2, space="PSUM") as ap_, \
                 tc.tile_pool(name="dp", bufs=2, space="PSUM") as dp, \
                 tc.tile_pool(name="op", bufs=1, space="PSUM") as op:
                for qc in range(NQC):
                    qsl = slice(qc * QC, (qc + 1) * QC)
                    at_qc = aq.tile([128, NQ, QC], BF, name="at_qc")
                    for h in range(NQ):
                        kvh = h // (NQ // NKV)
                        attn_ps = ap_.tile([128, QC], F32, name="attn_ps")
                        den_ps = dp.tile([1, QC], F32, name="den_ps")
                        for kc in range(KT):
                            s_ps = sp.tile([128, QC], F32, name="s_ps")
                            nc.tensor.matmul(
                                s_ps, k_sb[:, kvh, kc * 128:(kc + 1) * 128],
                                q_sb[:, h, qsl], start=True, stop=True)
                            e_t = ep.tile([128, QC], BF, name="e_t")
                            nc.scalar.activation(
                                out=e_t, in_=s_ps,
                                func=mybir.ActivationFunctionType.Exp, scale=scale)
                            nc.tensor.matmul(
                                attn_ps, v_sb[:, kc, kvh * 128:(kvh + 1) * 128], e_t,
                                start=(kc == 0), stop=(kc == KT - 1),
                                skip_group_check=True)
                            nc.tensor.matmul(
                                den_ps, ones_sb, e_t,
                                start=(kc == 0), stop=(kc == KT - 1),
                                skip_group_check=True)
                        rec = rb.tile([1, QC], F32, name="rec")
                        nc.vector.reciprocal(out=rec, in_=den_ps)
                        rec_bc = rb.tile([128, QC], F32, name="rec_bc")
                        nc.gpsimd.partition_broadcast(rec_bc, rec)
                        nc.vector.tensor_mul(at_qc[:, h, :], attn_ps, rec_bc)

                    # output projection for this q-chunk
                    for n in range(NO):
                        wos = wob.tile([128, NQ, 512], BF, name="wo_slab")
                        nc.sync.dma_start(
                            out=wos, in_=wo[:, n].rearrange("c p m -> p c m"))
                        o_t = ob.tile([128, QC // 128, 512], F32, name="o_t")
                        for tt in range(QC // 128):
                            ps_o = op.tile([128, 512], F32, name="ps_o")
                            for c in range(NQ):
                                nc.tensor.matmul(
                                    ps_o, at_qc[:, c, tt * 128:(tt + 1) * 128],
                                    wos[:, c, :],
                                    start=(c == 0), stop=(c == NQ - 1))
                            nc.scalar.activation(out=o_t[:, tt, :], in_=ps_o,
                                                 func=mybir.ActivationFunctionType.Copy)
                        nc.sync.dma_start(
                            out=o[qsl, n * 512:(n + 1) * 512].rearrange(
                                "(t p) m -> p t m", p=128),
                            in_=o_t)
    nc.compile()
    return nc


def _rope_tables(position_ids_b, S_):
    """cos/sin tables in [d=128, s] layout, sin sign-folded for the half-swap."""
    pos = position_ids_b.astype(np.float32)
    inv_freq = (1.0 / (ROPE_THETA ** (np.arange(0, HEAD_DIM, 2, dtype=np.float32)
                                      / HEAD_DIM))).astype(np.float32)
    freqs = pos[:, None] * inv_freq[None, :]          # [s, 64]
    emb = np.concatenate([freqs, freqs], axis=1)      # [s, 128]
    cos = np.cos(emb).T.copy()                        # [128, s]
    sin = np.sin(emb).T.copy()
    sin[:64] *= -1.0                                  # sign-fold for swap rope
    return cos.astype(BF_NP), sin.astype(BF_NP)


def _prep_core_inputs(hidden_states, position_ids, Wq, Wk, Wv, Wo):
    rmat = np.zeros((128, 128), dtype=np.float32)
    for i in range(128):
        rmat[i, (i + 64) % 128] = 1.0
    rmat = rmat.astype(BF_NP)

    HC = HIDDEN // 128
    in_maps = []
    for t in range(TP):
        fq = slice(1024 * t, 1024 * (t + 1))
        fkv = slice(256 * t, 256 * (t + 1))
        wq_t = np.ascontiguousarray(
            Wq[:, fq].reshape(HC, 128, 8, 128).transpose(0, 2, 1, 3)).astype(BF_NP)
        wk_t = np.ascontiguousarray(
            Wk[:, fkv].reshape(HC, 128, 2, 128).transpose(0, 2, 1, 3)).astype(BF_NP)
        wv_t = np.ascontiguousarray(Wv[:, fkv].reshape(HC, 128, 256)).astype(BF_NP)
        wo_t = np.ascontiguousarray(
            Wo[fq, :].reshape(8, 128, 8, 512).transpose(0, 2, 1, 3)).astype(BF_NP)
        for b in range(B):
            xt = np.ascontiguousarray(hidden_states[b].T).astype(BF_NP)
            cos, sin = _rope_tables(position_ids[b], S)
            in_maps.append({"xt": xt, "wq": wq_t, "wk": wk_t, "wv": wv_t,
                            "wo": wo_t, "cos": cos, "sin": sin, "rmat": rmat})
    return in_maps


_NC_CACHE = {}


def kernel(hidden_states, position_ids, Wq, Wk, Wv, Wo):
    if "nc" not in _NC_CACHE:
        _NC_CACHE["nc"] = build_nc(FULL_CFG)
    nc = _NC_CACHE["nc"]
    in_maps = _prep_core_inputs(np.asarray(hidden_states), np.asarray(position_ids),
                                np.asarray(Wq), np.asarray(Wk),
                                np.asarray(Wv), np.asarray(Wo))
    res = run_bass_kernel_spmd(nc, in_maps, core_ids=list(range(8)))
    out = np.zeros((B, S, HIDDEN), dtype=np.float32)
    for t in range(TP):
        for b in range(B):
            out[b] += res.results[t * B + b]["o"]
    return out
